# revision 41
# baseline (speedup 1.0000x reference)
"""DenseGAT layer on 8 trn2 NeuronCores — transposed-stationary formulation.

Math (per batch b, head t, query node i, source node j):
    z_ij = src_i + dst_j
    W_ij = adj_ij * exp(leakyrelu_0.2(z_ij));  out_i = (W @ h)_i / (W @ 1)_i

Identity: exp(lrelu(z)) = max(e^z, e^{0.2z}), each branch factorizes:
    e^z = e^{src_i} e^{dst_j},  e^{0.2z} = e^{0.2 src_i} e^{0.2 dst_j}
With M1 = 1[z>=0]*adj, b = e^{dst}, d = e^{0.2 dst}, r_i = e^{-0.8 src_i}
(the e^{src_i} row factor cancels in the softmax ratio):
    num   = M1@[b.h|b] + r * (adj@[d.h|d] - M1@[d.h|d])
    out_i = num[0:64] / num[64]

Kernel layout: everything lands i-MAJOR by using the masks/adj as matmul
STATIONARY ([128 j, 128 i] subblocks) and the value matrix V130 =
[b.h | b | d.h | d] (130 cols) as MOVING.  One accumulating pass per head
produces [U_b | den_b | U_d | den_d] at once ([128 i, 130] PSUM per
i-subblock); one shared 260-col pass gives the adj terms for all 4 heads.
LDWEIGHTS of the per-chunk stationaries is fully hidden behind matmuls
(measured 25ns cadence at 2-col moving).  The epilogue is i-major, so the
r_i combine / reciprocal / final scale are per-partition-scalar ops (DVE
reciprocal [128,1]; final multiply via ACT per-partition scale).

Mask build m1 = (src_i + dst_j >= 0) * adjT splits across engines:
DVE fused scalar_tensor_tensor (~1.2us/chunk) for most chunks; for the
rest ACT Sigmoid(1e9*z) (exactly {0,1}, verified) + gpsimd multiply.

h-phase runs in bf16 (4x faster than fp32 matmul); src/dst precision is
restored by sending x and P as bf16 value+residual pairs (xb+xr, Pb+Pr):
sd = xb@Pb + xr@Pb + xb@Pr accumulated in one fp32 PSUM region.

Sharding: core c -> batch c//4, query rows (c%4)*1024..+1024.
"""

import numpy as np
import ml_dtypes
from contextlib import ExitStack

import concourse.bass as bass
import concourse.mybir as mybir
import concourse.tile as tile
from concourse.bass import ts, ds
from concourse.bass_utils import run_bass_kernel_spmd
from concourse.masks import make_identity
from concourse.vector_clock import ScopedClock

B, N, IN = 2, 4096, 256
H, D = 4, 64
IBLK = 1024          # query rows per core
CH = N // 128        # 32 j-chunks
NSUB = IBLK // 128   # 8 i-subblocks per core
OCH = IBLK // 128    # own chunks (i-range) = 8
WCOL = IN + 16       # wtpb cols: [Wb 256 | Pb 8 | Pr 8]
V1 = D + 1           # 65: [value-cols | den-col]

F32 = mybir.dt.float32
BF16 = mybir.dt.bfloat16
FT = mybir.ActivationFunctionType
OP = mybir.AluOpType

LAST_RESULT = None  # BassKernelResults of the most recent run (for test harness)

# mask-build engine assignment per chunk (load balancing across DVE/ACT/GP):
#   GP set: st on ACT (sigmoid), mult on gpsimd
#   ACT-ST set: st on ACT (sigmoid), mult on DVE
#   rest: st + mult both on DVE
GP_SET = ()
ACTST_SET = (1, 2, 4, 5, 8, 10, 11, 13, 14)


def _install_drain_split(maxw=1):
    """This walrus build rejects instructions with more than ~2 sem waits
    ("Too many sync wait commands"). Tile's kernel-tail drain waits on every
    proc's final tick in a single instruction; split it into a chain of SP
    nops carrying one wait each."""
    if getattr(tile.TileContext, "_drain_split_installed", False):
        return

    def _split_drain_and_barrier(self, tick_clock, wait_clock):
        nc = self.nc
        probe = nc.sync.nop(nofuse=True)
        wait_clock.add_sem_waits(probe.ins, ScopedClock({None: tick_clock.global_clock}))
        si = probe.ins.sync_info
        waits = list(si.on_wait) if si is not None else []
        if len(waits) > maxw:
            probe.ins.sync_info = mybir.SyncInfo(
                on_wait=waits[:maxw], on_update=list(si.on_update)
            )
            for i in range(maxw, len(waits), maxw):
                extra = nc.sync.nop(nofuse=True)
                extra.ins.sync_info = mybir.SyncInfo(
                    on_wait=waits[i:i + maxw], on_update=[]
                )
        nc.sync.drain()
        nc.all_engine_barrier()
        assert self.sems is not None
        popped = nc._tile_sem_poison_stack.pop()
        assert popped is self._sem_poison
        nc.clear_and_free_semaphores(list(self.sems.allocated().values()))
        nc.all_engine_barrier()

    tile.TileContext._drain_and_barrier = _split_drain_and_barrier
    tile.TileContext._drain_split_installed = True


def _split_excess_waits(nc, maxw=1):
    """Move excess sem-waits (beyond maxw per instruction) onto same-engine
    NoOps inserted immediately before the instruction."""
    cnt = 0
    tpb = {mybir.EngineType.PE, mybir.EngineType.Activation, mybir.EngineType.Pool,
           mybir.EngineType.DVE, mybir.EngineType.SP}
    for f in nc.m.functions:
        for bb in f.blocks:
            out = []
            changed = False
            for inst in bb.instructions:
                si = getattr(inst, "sync_info", None)
                waits = list(si.on_wait) if si is not None else []
                if len(waits) > maxw and inst.engine in tpb:
                    changed = True
                    nlead = len(waits) - maxw
                    for k in range(0, nlead, maxw):
                        nop = mybir.InstNoOp(
                            name=f"wsplit{cnt}", engine=inst.engine, ins=[], outs=[],
                            sync_info=mybir.SyncInfo(
                                on_wait=waits[k:min(k + maxw, nlead)], on_update=[]))
                        cnt += 1
                        nc.register_instruction(nop, overwrite=True)
                        out.append(nop)
                    inst.sync_info = mybir.SyncInfo(
                        on_wait=waits[nlead:], on_update=list(si.on_update))
                out.append(inst)
            if changed:
                bb.instructions = out
    return cnt


def build_bass():
    _install_drain_split()
    nc = bass.Bass("TRN2", target_bir_lowering=False, debug=False, num_devices=1)

    adjT = nc.dram_tensor("adjT", [CH, 128, IBLK], BF16, kind="ExternalInput")
    xb = nc.dram_tensor("xb", [2, 128, N], BF16, kind="ExternalInput")
    xr = nc.dram_tensor("xr", [2, 128, N], BF16, kind="ExternalInput")
    xob = nc.dram_tensor("xob", [2, 128, IBLK], BF16, kind="ExternalInput")
    xor_ = nc.dram_tensor("xor", [2, 128, IBLK], BF16, kind="ExternalInput")
    wtpb = nc.dram_tensor("wtpb", [2, 128, WCOL], BF16, kind="ExternalInput")
    outT = nc.dram_tensor("outT", [IBLK, H * D], BF16, kind="ExternalOutput")

    with ExitStack() as ctx:
        tc = ctx.enter_context(tile.TileContext(nc))
        const = ctx.enter_context(tc.tile_pool(name="const", bufs=1))

        ident = const.tile([128, 128], F32, tag="ident")
        make_identity(nc, ident[:])

        adjT_sb = const.tile([128, CH, IBLK], BF16, tag="adjT")

        # value matrix per (chunk, head): [b.h | b | d.h | d] (130 cols)
        V = const.tile([128, CH, H, 2 * V1], BF16, tag="V")
        sd_sb = const.tile([128, CH, 8], F32, tag="sd")       # [src 0:4 | dst 4:8]
        ndst = const.tile([128, CH, H], F32, tag="ndst")      # -dst (DVE is_ge scalar)
        pdst9 = const.tile([128, CH, H], F32, tag="pdst9")    # +1e9*dst (ACT bias)
        bcol = const.tile([128, CH, H, 1], F32, tag="bcol")   # e^dst
        dcol = const.tile([128, CH, H, 1], F32, tag="dcol")   # e^{0.2 dst}
        r_sb = const.tile([128, OCH, H], F32, tag="r")        # e^{-0.8 src_own}
        sbb = const.tile([128, H, IBLK], BF16, tag="sbb")     # src_i bcast over j
        srowT = const.tile([32, 128], BF16, tag="srowT")
        A_sb = const.tile([128, NSUB, H * V1], F32, tag="A")  # adj-pass results

        def bcast(dst_ap, src_row_ap):
            # DMA-broadcast one SBUF row across partitions: the repeat is a
            # stride-0 *free* dim on the source, iterated in the same order as
            # the dest's partition dim so the element streams line up.
            lay = [list(src_row_ap.ap[0]), [0, dst_ap.shape[0]]] + [
                list(dims) for dims in src_row_ap.ap[1:]]
            src_b = bass.AP(src_row_ap.tensor, src_row_ap.offset, lay)
            nc.sync.dma_start(dst_ap, src_b)

        # mask pools live across phase A and the masked passes so head-0
        # masks can pre-fill the m1 queue while the h-phase still runs
        m1p = ctx.enter_context(tc.tile_pool(name="m1p", bufs=12))
        stp = ctx.enter_context(tc.tile_pool(name="stp", bufs=4))
        m1_pre = {}

        def emit_mask(t, c):
            m1 = m1p.tile([128, IBLK], BF16, tag="m1")
            kind = (c + 3 * t) % 16
            if kind in GP_SET or kind in ACTST_SET:
                st = stp.tile([128, IBLK], BF16, tag="st")
                nc.scalar.activation(st[:], sbb[:, t, :], FT.Sigmoid,
                                     bias=pdst9[:, c, t:t + 1], scale=1e9)
                if kind in GP_SET:
                    nc.gpsimd.tensor_mul(m1[:], st[:], adjT_sb[:, c, :])
                else:
                    nc.vector.tensor_mul(m1[:], st[:], adjT_sb[:, c, :])
            else:
                nc.vector.scalar_tensor_tensor(m1[:], sbb[:, t, :],
                                               ndst[:, c, t:t + 1],
                                               adjT_sb[:, c, :],
                                               OP.is_ge, OP.mult)
            return m1

        # ---------------- phase A: h projection + src/dst ----------------
        with (
            tc.tile_pool(name="xin", bufs=1) as xin,
            tc.tile_pool(name="pps", bufs=3, space="PSUM") as pps,
            tc.tile_pool(name="ppt", bufs=1, space="PSUM") as ppt,
        ):
            xb_sb = [xin.tile([128, N], BF16, tag=f"xb{k}", name=f"xbsb{k}") for k in range(2)]
            xr_sb = [xin.tile([128, N], BF16, tag=f"xr{k}", name=f"xrsb{k}") for k in range(2)]
            xob_sb = [xin.tile([128, IBLK], BF16, tag=f"xob{k}", name=f"xobsb{k}") for k in range(2)]
            xor_sb = [xin.tile([128, IBLK], BF16, tag=f"xor{k}", name=f"xorsb{k}") for k in range(2)]
            wtp_sb = [xin.tile([128, WCOL], BF16, tag=f"wt{k}", name=f"wtsb{k}") for k in range(2)]
            # smallest inputs first: sdo/sbb come up within ~8us, h-phase
            # right after; adjT streams behind, hidden under phase A
            for k in range(2):
                nc.sync.dma_start(wtp_sb[k][:], wtpb.ap()[k])
                nc.sync.dma_start(xob_sb[k][:], xob.ap()[k])
                nc.sync.dma_start(xor_sb[k][:], xor_.ap()[k])
            for k in range(2):
                nc.sync.dma_start(xb_sb[k][:], xb.ap()[k])
                nc.sync.dma_start(xr_sb[k][:], xr.ap()[k])
            for c in range(CH):
                nc.sync.dma_start(adjT_sb[:, c, :], adjT.ap()[c])

            # own-row src (fp32-compensated, q-independent via xob/xor inputs)
            sdo_sb = xin.tile([128, OCH, 4], F32, tag="sdo")
            for oc in range(OCH):
                pho = pps.tile([128, 4], F32, tag="pho")
                for k in range(2):
                    nc.tensor.matmul(pho[:], xob_sb[k][:, ts(oc, 128)],
                                     wtp_sb[k][:, IN:IN + 4], start=(k == 0), stop=False)
                    nc.tensor.matmul(pho[:], xob_sb[k][:, ts(oc, 128)],
                                     wtp_sb[k][:, IN + 8:IN + 12], start=False, stop=False)
                    nc.tensor.matmul(pho[:], xor_sb[k][:, ts(oc, 128)],
                                     wtp_sb[k][:, IN:IN + 4], start=False, stop=(k == 1))
                nc.scalar.copy(sdo_sb[:, oc, :], pho[:])
                # r_i = e^{-0.8 src} for own chunks, i-major
                nc.scalar.activation(r_sb[:, oc, :], sdo_sb[:, oc, :], FT.Exp, scale=-0.8)

            # srow: own-chunk src columns -> [32, 128] rows via PE transpose
            stile = xin.tile([128, 32], F32, tag="stile")
            for t in range(H):
                nc.scalar.copy(stile[:, ts(t, OCH)], sdo_sb[:, :, t])
            pst = ppt.tile([32, 128], F32, tag="pst")
            nc.tensor.transpose(pst[:], stile[:], ident[:])
            nc.scalar.copy(srowT[:], pst[:])
            for t in range(H):
                for oc in range(OCH):
                    bcast(sbb[:, t, ts(oc, 128)], srowT[t * OCH + oc:t * OCH + oc + 1, :])

            for c in range(CH):
                ph = pps.tile([128, IN + 8], F32, tag="ph")
                # sd corrections first, full-region matmul last so the whole
                # accumulation group gets a proper stop
                nc.tensor.matmul(ph[:], xb_sb[0][:, ts(c, 128)],
                                 wtp_sb[0][:, 0:IN + 8], start=True, stop=False)
                for k in range(2):
                    # sd += xb@Pr (Pr cols into the same 256:264 psum region)
                    nc.tensor.matmul(ph[:, IN:IN + 8], xb_sb[k][:, ts(c, 128)],
                                     wtp_sb[k][:, IN + 8:IN + 16],
                                     start=False, stop=False)
                    # sd += xr@Pb
                    nc.tensor.matmul(ph[:, IN:IN + 8], xr_sb[k][:, ts(c, 128)],
                                     wtp_sb[k][:, IN:IN + 8],
                                     start=False, stop=False)
                nc.tensor.matmul(ph[:], xb_sb[1][:, ts(c, 128)],
                                 wtp_sb[1][:, 0:IN + 8], start=False, stop=True)
                # drains: sd on ACT + per-chunk exps; V built straight from
                # PSUM on DVE (b-branch) — gp can't read PSUM, so d-branch too
                nc.scalar.copy(sd_sb[:, c, :], ph[:, IN:IN + 8])
                nc.scalar.activation(bcol[:, c, :, 0], sd_sb[:, c, 4:8], FT.Exp)
                nc.scalar.activation(dcol[:, c, :, 0], sd_sb[:, c, 4:8],
                                     FT.Exp, scale=0.2)
                php = ph[:, 0:IN]
                hview = bass.AP(php.tensor, php.offset,
                                [list(php.ap[0]), [D, H], [1, D]])
                _, cb = bass.broadcast_tensor_aps(
                    V[:, c, :, 0:D], bcol[:, c, :, :])
                nc.vector.tensor_tensor(V[:, c, :, 0:D], hview, cb, OP.mult)
                _, cd = bass.broadcast_tensor_aps(
                    V[:, c, :, V1:V1 + D], dcol[:, c, :, :])
                nc.vector.tensor_tensor(V[:, c, :, V1:V1 + D], hview, cd, OP.mult)
                if c % 8 == 7:
                    g = ds(c - 7, 8)
                    nc.scalar.activation(ndst[:, g, :], sd_sb[:, g, 4:8],
                                         FT.Copy, scale=-1.0)
                    nc.scalar.activation(pdst9[:, g, :], sd_sb[:, g, 4:8],
                                         FT.Copy, scale=1e9)
                    for t in range(H):
                        nc.scalar.copy(V[:, g, t, D], bcol[:, g, t, 0])
                        nc.scalar.copy(V[:, g, t, V1 + D], dcol[:, g, t, 0])
                    for cc in range(c - 7, c + 1):
                        if cc < 10:  # leave m1-pool slack so later phase-A
                            m1_pre[(0, cc)] = emit_mask(0, cc)  # DVE work isn't blocked

        # ------------- masked passes per head (adj rides with head 0) ------
        with (
            tc.tile_pool(name="pm", bufs=1, space="PSUM") as pm,
            tc.tile_pool(name="psb", bufs=2) as psb,
            tc.tile_pool(name="epp", bufs=2) as epp,
        ):
            def epilogue(t, Ps):
                # batched where the scalar is shared; per-s where r/rec differ
                t1 = epp.tile([128, NSUB, V1], F32, tag="t1")
                nc.gpsimd.tensor_tensor(t1[:], A_sb[:, :, ts(t, V1)],
                                        Ps[:, :, V1:2 * V1], OP.subtract)
                for s in range(NSUB):
                    nc.scalar.activation(t1[:, s, :], t1[:, s, :], FT.Copy,
                                         scale=r_sb[:, s, t:t + 1])
                o2 = epp.tile([128, NSUB, V1], F32, tag="o2")
                nc.vector.tensor_tensor(o2[:], t1[:], Ps[:, :, 0:V1], OP.add)
                rec = epp.tile([128, NSUB], F32, tag="rec")
                nc.vector.reciprocal(rec[:], o2[:, :, D])
                for s in range(NSUB):
                    of = epp.tile([128, D], BF16, tag="of")
                    nc.scalar.activation(of[:], o2[:, s, 0:D], FT.Copy,
                                         scale=rec[:, s:s + 1])
                    nc.sync.dma_start(outT.ap()[ds(s * 128, 128), ts(t, D)], of[:])

            prev = None  # (t, Ps) pending epilogue, emitted a few chunks into
            # the next head so its ops never block that head's mask queue
            for t in range(H):
                # P tile: 8 subblocks, one full PSUM bank (2KB zero-region)
                # each; masked pass in cols 0:130, and for head 0 the shared
                # adj pass accumulates in cols 130:390 of the same banks
                P = pm.tile([128, NSUB, 512], F32, tag="P")
                for c in range(CH):
                    m1 = m1_pre.pop((t, c), None)
                    if m1 is None:
                        m1 = emit_mask(t, c)
                    for s in range(NSUB):
                        nc.tensor.matmul(P[:, s, 0:2 * V1], m1[:, ts(s, 128)],
                                         V[:, c, t, :],
                                         start=(c == 0), stop=(c == CH - 1))
                        if t == 0:
                            nc.tensor.matmul(P[:, s, 2 * V1:6 * V1],
                                             adjT_sb[:, c, ts(s, 128)],
                                             V[:, c, :, V1:2 * V1],
                                             start=False, stop=False,
                                             skip_group_check=True)
                    if prev is not None and c == 3:
                        epilogue(*prev)
                        prev = None
                # drain PSUM fast (frees P for the next head)
                Ps = psb.tile([128, NSUB, 2 * V1], F32, tag="Ps")
                if t == 0:
                    nc.scalar.copy(A_sb[:, :, :], P[:, :, 2 * V1:6 * V1])
                nc.scalar.copy(Ps[:, :, :], P[:, :, 0:2 * V1])
                prev = (t, Ps)
            epilogue(*prev)
    _split_excess_waits(nc)
    return nc


_CACHED = None


def _get_bass():
    global _CACHED
    if _CACHED is None:
        _CACHED = build_bass()
    return _CACHED


def _prep_inputs(x, adj, W_proj, attn_src, attn_dst):
    bf = ml_dtypes.bfloat16
    A_blk = np.zeros((IN, 2 * H), np.float32)
    for t in range(H):
        A_blk[t * D:(t + 1) * D, t] = attn_src[t]
        A_blk[t * D:(t + 1) * D, H + t] = attn_dst[t]
    P = W_proj.T.astype(np.float32) @ A_blk                      # [256, 8]
    Wb = W_proj.T.astype(bf)
    Pb = P.astype(bf)
    Pr = (P - Pb.astype(np.float32)).astype(bf)
    wtpb_full = np.concatenate(
        [Wb.astype(np.float32), Pb.astype(np.float32), Pr.astype(np.float32)],
        axis=1).astype(bf)
    wtpb_c = np.ascontiguousarray(wtpb_full.reshape(2, 128, WCOL))

    in_maps = []
    xcache = {}
    for core in range(8):
        b, qq = core // 4, core % 4
        i0 = qq * IBLK
        if b not in xcache:
            xT = np.ascontiguousarray(x[b].T)                    # [256, 4096] f32
            xb_f = xT.astype(bf)
            xr_f = (xT - xb_f.astype(np.float32)).astype(bf)
            xcache[b] = (xT, xb_f, xr_f)
        xT, xb_f, xr_f = xcache[b]
        adjT_c = np.ascontiguousarray(adj[b, i0:i0 + IBLK, :].T.astype(bf))
        in_maps.append({
            "adjT": adjT_c.reshape(CH, 128, IBLK),
            "xb": np.ascontiguousarray(xb_f).reshape(2, 128, N),
            "xr": np.ascontiguousarray(xr_f).reshape(2, 128, N),
            "xob": np.ascontiguousarray(xb_f[:, i0:i0 + IBLK]).reshape(2, 128, IBLK),
            "xor": np.ascontiguousarray(xr_f[:, i0:i0 + IBLK]).reshape(2, 128, IBLK),
            "wtpb": wtpb_c,
        })
    return in_maps


def kernel(x, adj, W_proj, attn_src, attn_dst):
    global LAST_RESULT
    x = np.asarray(x, np.float32)
    adj = np.asarray(adj)
    W_proj = np.asarray(W_proj, np.float32)
    attn_src = np.asarray(attn_src, np.float32)
    attn_dst = np.asarray(attn_dst, np.float32)

    nc = _get_bass()
    in_maps = _prep_inputs(x, adj, W_proj, attn_src, attn_dst)
    br = run_bass_kernel_spmd(nc, in_maps, core_ids=list(range(8)))
    LAST_RESULT = br

    out = np.empty((B, N, H * D), np.float32)
    for core in range(8):
        b, qq = core // 4, core % 4
        i0 = qq * IBLK
        out[b, i0:i0 + IBLK, :] = br.results[core]["outT"].astype(np.float32)
    return out


# revision 43
# speedup vs baseline: 1.0566x; 1.0566x over previous
"""DenseGAT layer on 8 trn2 NeuronCores — transposed-stationary formulation.

Math (per batch b, head t, query node i, source node j):
    z_ij = src_i + dst_j
    W_ij = adj_ij * exp(leakyrelu_0.2(z_ij));  out_i = (W @ h)_i / (W @ 1)_i

Identity: exp(lrelu(z)) = max(e^z, e^{0.2z}), each branch factorizes:
    e^z = e^{src_i} e^{dst_j},  e^{0.2z} = e^{0.2 src_i} e^{0.2 dst_j}
With M1 = 1[z>=0]*adj, b = e^{dst}, d = e^{0.2 dst}, r_i = e^{-0.8 src_i}
(the e^{src_i} row factor cancels in the softmax ratio):
    num   = M1@[b.h|b] + r * (adj@[d.h|d] - M1@[d.h|d])
    out_i = num[0:64] / num[64]

Kernel layout: everything lands i-MAJOR by using the masks/adj as matmul
STATIONARY ([128 j, 128 i] subblocks) and the value matrix V130 =
[b.h | b | d.h | d] (130 cols) as MOVING.  One accumulating pass per head
produces [U_b | den_b | U_d | den_d] at once ([128 i, 130] PSUM per
i-subblock); one shared 260-col pass gives the adj terms for all 4 heads.
LDWEIGHTS of the per-chunk stationaries is fully hidden behind matmuls
(measured 25ns cadence at 2-col moving).  The epilogue is i-major, so the
r_i combine / reciprocal / final scale are per-partition-scalar ops (DVE
reciprocal [128,1]; final multiply via ACT per-partition scale).

Mask build m1 = (src_i + dst_j >= 0) * adjT splits across engines:
DVE fused scalar_tensor_tensor (~1.2us/chunk) for most chunks; for the
rest ACT Sigmoid(1e9*z) (exactly {0,1}, verified) + gpsimd multiply.

h-phase runs in bf16 (4x faster than fp32 matmul); src/dst precision is
restored by sending x and P as bf16 value+residual pairs (xb+xr, Pb+Pr):
sd = xb@Pb + xr@Pb + xb@Pr accumulated in one fp32 PSUM region.

Sharding: core c -> batch c//4, query rows (c%4)*1024..+1024.
"""

import numpy as np
import ml_dtypes
from contextlib import ExitStack

import concourse.bass as bass
import concourse.mybir as mybir
import concourse.tile as tile
from concourse.bass import ts, ds
from concourse.bass_utils import run_bass_kernel_spmd
from concourse.masks import make_identity
from concourse.vector_clock import ScopedClock

B, N, IN = 2, 4096, 256
H, D = 4, 64
IBLK = 1024          # query rows per core
CH = N // 128        # 32 j-chunks
NSUB = IBLK // 128   # 8 i-subblocks per core
OCH = IBLK // 128    # own chunks (i-range) = 8
WCOL = IN + 16       # wtpb cols: [Wb 256 | Pb 8 | Pr 8]
V1 = D + 1           # 65: [value-cols | den-col]

F32 = mybir.dt.float32
BF16 = mybir.dt.bfloat16
FT = mybir.ActivationFunctionType
OP = mybir.AluOpType

LAST_RESULT = None  # BassKernelResults of the most recent run (for test harness)

# mask-build engine assignment per chunk (load balancing across DVE/ACT/GP):
#   GP set: st on ACT (sigmoid), mult on gpsimd
#   ACT-ST set: st on ACT (sigmoid), mult on DVE
#   rest: st + mult both on DVE
GP_SET = ()
ACTST_SET = (1, 2, 4, 5, 8, 10, 11, 13, 14)


def _install_drain_split(maxw=1):
    """This walrus build rejects instructions with more than ~2 sem waits
    ("Too many sync wait commands"). Tile's kernel-tail drain waits on every
    proc's final tick in a single instruction; split it into a chain of SP
    nops carrying one wait each."""
    if getattr(tile.TileContext, "_drain_split_installed", False):
        return

    def _split_drain_and_barrier(self, tick_clock, wait_clock):
        nc = self.nc
        probe = nc.sync.nop(nofuse=True)
        wait_clock.add_sem_waits(probe.ins, ScopedClock({None: tick_clock.global_clock}))
        si = probe.ins.sync_info
        waits = list(si.on_wait) if si is not None else []
        if len(waits) > maxw:
            probe.ins.sync_info = mybir.SyncInfo(
                on_wait=waits[:maxw], on_update=list(si.on_update)
            )
            for i in range(maxw, len(waits), maxw):
                extra = nc.sync.nop(nofuse=True)
                extra.ins.sync_info = mybir.SyncInfo(
                    on_wait=waits[i:i + maxw], on_update=[]
                )
        nc.sync.drain()
        nc.all_engine_barrier()
        assert self.sems is not None
        popped = nc._tile_sem_poison_stack.pop()
        assert popped is self._sem_poison
        nc.clear_and_free_semaphores(list(self.sems.allocated().values()))
        nc.all_engine_barrier()

    tile.TileContext._drain_and_barrier = _split_drain_and_barrier
    tile.TileContext._drain_split_installed = True


def _split_excess_waits(nc, maxw=1):
    """Move excess sem-waits (beyond maxw per instruction) onto same-engine
    NoOps inserted immediately before the instruction."""
    cnt = 0
    tpb = {mybir.EngineType.PE, mybir.EngineType.Activation, mybir.EngineType.Pool,
           mybir.EngineType.DVE, mybir.EngineType.SP}
    for f in nc.m.functions:
        for bb in f.blocks:
            out = []
            changed = False
            for inst in bb.instructions:
                si = getattr(inst, "sync_info", None)
                waits = list(si.on_wait) if si is not None else []
                if len(waits) > maxw and inst.engine in tpb:
                    changed = True
                    nlead = len(waits) - maxw
                    for k in range(0, nlead, maxw):
                        nop = mybir.InstNoOp(
                            name=f"wsplit{cnt}", engine=inst.engine, ins=[], outs=[],
                            sync_info=mybir.SyncInfo(
                                on_wait=waits[k:min(k + maxw, nlead)], on_update=[]))
                        cnt += 1
                        nc.register_instruction(nop, overwrite=True)
                        out.append(nop)
                    inst.sync_info = mybir.SyncInfo(
                        on_wait=waits[nlead:], on_update=list(si.on_update))
                out.append(inst)
            if changed:
                bb.instructions = out
    return cnt


def build_bass():
    _install_drain_split()
    nc = bass.Bass("TRN2", target_bir_lowering=False, debug=False, num_devices=1)

    adjT = nc.dram_tensor("adjT", [CH, 128, IBLK], BF16, kind="ExternalInput")
    xb = nc.dram_tensor("xb", [2, 128, N], BF16, kind="ExternalInput")
    xr = nc.dram_tensor("xr", [2, 128, N], BF16, kind="ExternalInput")
    xob = nc.dram_tensor("xob", [2, 128, IBLK], BF16, kind="ExternalInput")
    xor_ = nc.dram_tensor("xor", [2, 128, IBLK], BF16, kind="ExternalInput")
    wtpb = nc.dram_tensor("wtpb", [2, 128, WCOL], BF16, kind="ExternalInput")
    outT = nc.dram_tensor("outT", [IBLK, H * D], BF16, kind="ExternalOutput")

    with ExitStack() as ctx:
        tc = ctx.enter_context(tile.TileContext(nc))
        const = ctx.enter_context(tc.tile_pool(name="const", bufs=1))

        ident = const.tile([128, 128], F32, tag="ident")
        make_identity(nc, ident[:])

        adjT_sb = const.tile([128, CH, IBLK], BF16, tag="adjT")

        # value matrix per (chunk, head): [b.h | b | d.h | d] (130 cols)
        V = const.tile([128, CH, H, 2 * V1], BF16, tag="V")
        sd_sb = const.tile([128, CH, 8], F32, tag="sd")       # [src 0:4 | dst 4:8]
        ndst = const.tile([128, CH, H], F32, tag="ndst")      # -dst (DVE is_ge scalar)
        pdst9 = const.tile([128, CH, H], F32, tag="pdst9")    # +1e9*dst (ACT bias)
        bcol = const.tile([128, CH, H, 1], F32, tag="bcol")   # e^dst
        dcol = const.tile([128, CH, H, 1], F32, tag="dcol")   # e^{0.2 dst}
        r_sb = const.tile([128, OCH, H], F32, tag="r")        # e^{-0.8 src_own}
        sbb = const.tile([128, H, IBLK], BF16, tag="sbb")     # src_i bcast over j
        srowT = const.tile([32, 128], BF16, tag="srowT")
        A_sb = const.tile([128, NSUB, H * V1], F32, tag="A")  # adj-pass results

        def bcast(dst_ap, src_row_ap):
            # DMA-broadcast one SBUF row across partitions: the repeat is a
            # stride-0 *free* dim on the source, iterated in the same order as
            # the dest's partition dim so the element streams line up.
            lay = [list(src_row_ap.ap[0]), [0, dst_ap.shape[0]]] + [
                list(dims) for dims in src_row_ap.ap[1:]]
            src_b = bass.AP(src_row_ap.tensor, src_row_ap.offset, lay)
            nc.sync.dma_start(dst_ap, src_b)

        # mask pools live across phase A and the masked passes so head-0
        # masks can pre-fill the m1 queue while the h-phase still runs
        m1p = ctx.enter_context(tc.tile_pool(name="m1p", bufs=6))
        m2p = ctx.enter_context(tc.tile_pool(name="m2p", bufs=3))
        stp = ctx.enter_context(tc.tile_pool(name="stp", bufs=2))
        st2p = ctx.enter_context(tc.tile_pool(name="st2p", bufs=2))
        m1_pre = {}

        def head_plan(t):
            # group adjacent ACTST chunks in pairs: their DVE multiply has no
            # per-chunk scalar, so one double-wide op covers both
            groups, c = [], 0
            while c < CH:
                k, k2 = (c + 3 * t) % 16, (c + 1 + 3 * t) % 16
                if c + 1 < CH and k in ACTST_SET and k2 in ACTST_SET:
                    groups.append((c, 2)); c += 2
                else:
                    groups.append((c, 1)); c += 1
            return groups

        def emit_mask(t, c, w):
            if w == 2:
                st2 = st2p.tile([128, 2, IBLK], BF16, tag="st2")
                for j in range(2):
                    nc.scalar.activation(st2[:, j, :], sbb[:, t, :], FT.Sigmoid,
                                         bias=pdst9[:, c + j, t:t + 1], scale=1e9)
                m1 = m2p.tile([128, 2, IBLK], BF16, tag="m2")
                nc.vector.tensor_mul(m1[:], st2[:], adjT_sb[:, c:c + 2, :])
                return m1
            m1 = m1p.tile([128, 1, IBLK], BF16, tag="m1")
            kind = (c + 3 * t) % 16
            if kind in ACTST_SET:
                st = stp.tile([128, IBLK], BF16, tag="st")
                nc.scalar.activation(st[:], sbb[:, t, :], FT.Sigmoid,
                                     bias=pdst9[:, c, t:t + 1], scale=1e9)
                nc.vector.tensor_mul(m1[:, 0, :], st[:], adjT_sb[:, c, :])
            else:
                nc.vector.scalar_tensor_tensor(m1[:, 0, :], sbb[:, t, :],
                                               ndst[:, c, t:t + 1],
                                               adjT_sb[:, c, :],
                                               OP.is_ge, OP.mult)
            return m1

        # ---------------- phase A: h projection + src/dst ----------------
        with (
            tc.tile_pool(name="xin", bufs=1) as xin,
            tc.tile_pool(name="pps", bufs=3, space="PSUM") as pps,
            tc.tile_pool(name="ppt", bufs=1, space="PSUM") as ppt,
        ):
            xb_sb = [xin.tile([128, N], BF16, tag=f"xb{k}", name=f"xbsb{k}") for k in range(2)]
            xr_sb = [xin.tile([128, N], BF16, tag=f"xr{k}", name=f"xrsb{k}") for k in range(2)]
            xob_sb = [xin.tile([128, IBLK], BF16, tag=f"xob{k}", name=f"xobsb{k}") for k in range(2)]
            xor_sb = [xin.tile([128, IBLK], BF16, tag=f"xor{k}", name=f"xorsb{k}") for k in range(2)]
            wtp_sb = [xin.tile([128, WCOL], BF16, tag=f"wt{k}", name=f"wtsb{k}") for k in range(2)]
            # smallest inputs first: sdo/sbb come up within ~8us, h-phase
            # right after; adjT streams behind, hidden under phase A
            for k in range(2):
                nc.sync.dma_start(wtp_sb[k][:], wtpb.ap()[k])
                nc.sync.dma_start(xob_sb[k][:], xob.ap()[k])
                nc.sync.dma_start(xor_sb[k][:], xor_.ap()[k])
            for k in range(2):
                nc.sync.dma_start(xb_sb[k][:], xb.ap()[k])
                nc.sync.dma_start(xr_sb[k][:], xr.ap()[k])
            for c in range(CH):
                nc.sync.dma_start(adjT_sb[:, c, :], adjT.ap()[c])

            # own-row src (fp32-compensated, q-independent via xob/xor inputs)
            sdo_sb = xin.tile([128, OCH, 4], F32, tag="sdo")
            for oc in range(OCH):
                pho = pps.tile([128, 4], F32, tag="pho")
                for k in range(2):
                    nc.tensor.matmul(pho[:], xob_sb[k][:, ts(oc, 128)],
                                     wtp_sb[k][:, IN:IN + 4], start=(k == 0), stop=False)
                    nc.tensor.matmul(pho[:], xob_sb[k][:, ts(oc, 128)],
                                     wtp_sb[k][:, IN + 8:IN + 12], start=False, stop=False)
                    nc.tensor.matmul(pho[:], xor_sb[k][:, ts(oc, 128)],
                                     wtp_sb[k][:, IN:IN + 4], start=False, stop=(k == 1))
                nc.scalar.copy(sdo_sb[:, oc, :], pho[:])
                # r_i = e^{-0.8 src} for own chunks, i-major
                nc.scalar.activation(r_sb[:, oc, :], sdo_sb[:, oc, :], FT.Exp, scale=-0.8)

            # srow: own-chunk src columns -> [32, 128] rows via PE transpose
            stile = xin.tile([128, 32], F32, tag="stile")
            for t in range(H):
                nc.vector.tensor_copy(stile[:, ts(t, OCH)], sdo_sb[:, :, t])
            pst = ppt.tile([32, 128], F32, tag="pst")
            nc.tensor.transpose(pst[:], stile[:], ident[:])
            nc.scalar.copy(srowT[:], pst[:])
            for t in range(H):
                for oc in range(OCH):
                    bcast(sbb[:, t, ts(oc, 128)], srowT[t * OCH + oc:t * OCH + oc + 1, :])

            for c in range(CH):
                ph = pps.tile([128, IN + 8], F32, tag="ph")
                # sd corrections first, full-region matmul last so the whole
                # accumulation group gets a proper stop
                nc.tensor.matmul(ph[:], xb_sb[0][:, ts(c, 128)],
                                 wtp_sb[0][:, 0:IN + 8], start=True, stop=False)
                for k in range(2):
                    # sd += xb@Pr (Pr cols into the same 256:264 psum region)
                    nc.tensor.matmul(ph[:, IN:IN + 8], xb_sb[k][:, ts(c, 128)],
                                     wtp_sb[k][:, IN + 8:IN + 16],
                                     start=False, stop=False)
                    # sd += xr@Pb
                    nc.tensor.matmul(ph[:, IN:IN + 8], xr_sb[k][:, ts(c, 128)],
                                     wtp_sb[k][:, IN:IN + 8],
                                     start=False, stop=False)
                nc.tensor.matmul(ph[:], xb_sb[1][:, ts(c, 128)],
                                 wtp_sb[1][:, 0:IN + 8], start=False, stop=True)
                # drains: sd on ACT + per-chunk exps; V built straight from
                # PSUM on DVE (b-branch) — gp can't read PSUM, so d-branch too
                nc.scalar.copy(sd_sb[:, c, :], ph[:, IN:IN + 8])
                nc.scalar.activation(bcol[:, c, :, 0], sd_sb[:, c, 4:8], FT.Exp)
                nc.scalar.activation(dcol[:, c, :, 0], sd_sb[:, c, 4:8],
                                     FT.Exp, scale=0.2)
                php = ph[:, 0:IN]
                hview = bass.AP(php.tensor, php.offset,
                                [list(php.ap[0]), [D, H], [1, D]])
                _, cb = bass.broadcast_tensor_aps(
                    V[:, c, :, 0:D], bcol[:, c, :, :])
                nc.vector.tensor_tensor(V[:, c, :, 0:D], hview, cb, OP.mult)
                _, cd = bass.broadcast_tensor_aps(
                    V[:, c, :, V1:V1 + D], dcol[:, c, :, :])
                nc.vector.tensor_tensor(V[:, c, :, V1:V1 + D], hview, cd, OP.mult)
                if c % 8 == 7:
                    g = ds(c - 7, 8)
                    nc.scalar.activation(ndst[:, g, :], sd_sb[:, g, 4:8],
                                         FT.Copy, scale=-1.0)
                    nc.scalar.activation(pdst9[:, g, :], sd_sb[:, g, 4:8],
                                         FT.Copy, scale=1e9)
                    for t in range(H):
                        nc.vector.tensor_copy(V[:, g, t, D], bcol[:, g, t, 0])
                        nc.vector.tensor_copy(V[:, g, t, V1 + D], dcol[:, g, t, 0])
                    if c == 7:  # pre-fill head-0 masks for chunks 0..7
                        for (gc, gw) in head_plan(0):
                            if gc + gw <= 8:
                                m1_pre[(0, gc)] = emit_mask(0, gc, gw)

        # ------------- masked passes per head (adj rides with head 0) ------
        with (
            tc.tile_pool(name="pm", bufs=1, space="PSUM") as pm,
            tc.tile_pool(name="psb", bufs=2) as psb,
            tc.tile_pool(name="epp", bufs=2) as epp,
        ):
            def epilogue(t, Ps):
                # batched where the scalar is shared; per-s where r/rec differ
                t1 = epp.tile([128, NSUB, V1], F32, tag="t1")
                nc.gpsimd.tensor_tensor(t1[:], A_sb[:, :, ts(t, V1)],
                                        Ps[:, :, V1:2 * V1], OP.subtract)
                for s in range(NSUB):
                    nc.scalar.activation(t1[:, s, :], t1[:, s, :], FT.Copy,
                                         scale=r_sb[:, s, t:t + 1])
                o2 = epp.tile([128, NSUB, V1], F32, tag="o2")
                nc.vector.tensor_tensor(o2[:], t1[:], Ps[:, :, 0:V1], OP.add)
                rec = epp.tile([128, NSUB], F32, tag="rec")
                nc.vector.reciprocal(rec[:], o2[:, :, D])
                for s in range(NSUB):
                    of = epp.tile([128, D], BF16, tag="of")
                    nc.scalar.activation(of[:], o2[:, s, 0:D], FT.Copy,
                                         scale=rec[:, s:s + 1])
                    nc.sync.dma_start(outT.ap()[ds(s * 128, 128), ts(t, D)], of[:])

            prev = None  # (t, Ps) pending epilogue, emitted a few chunks into
            # the next head so its ops never block that head's mask queue
            for t in range(H):
                # P tile: 8 subblocks, one full PSUM bank (2KB zero-region)
                # each; masked pass in cols 0:130, and for head 0 the shared
                # adj pass accumulates in cols 130:390 of the same banks
                P = pm.tile([128, NSUB, 512], F32, tag="P")
                for (gc, gw) in head_plan(t):
                    m1 = m1_pre.pop((t, gc), None)
                    if m1 is None:
                        m1 = emit_mask(t, gc, gw)
                    for c in range(gc, gc + gw):
                        for s in range(NSUB):
                            nc.tensor.matmul(P[:, s, 0:2 * V1],
                                             m1[:, c - gc, ts(s, 128)],
                                             V[:, c, t, :],
                                             start=(c == 0), stop=(c == CH - 1))
                            if t == 0:
                                nc.tensor.matmul(P[:, s, 2 * V1:6 * V1],
                                                 adjT_sb[:, c, ts(s, 128)],
                                                 V[:, c, :, V1:2 * V1],
                                                 start=False, stop=False,
                                                 skip_group_check=True)
                        if prev is not None and c == 3:
                            epilogue(*prev)
                            prev = None
                # drain PSUM fast (frees P for the next head)
                Ps = psb.tile([128, NSUB, 2 * V1], F32, tag="Ps")
                if t == 0:
                    nc.scalar.copy(A_sb[:, :, :], P[:, :, 2 * V1:6 * V1])
                nc.scalar.copy(Ps[:, :, :], P[:, :, 0:2 * V1])
                prev = (t, Ps)
            epilogue(*prev)
    _split_excess_waits(nc)
    return nc


_CACHED = None


def _get_bass():
    global _CACHED
    if _CACHED is None:
        _CACHED = build_bass()
    return _CACHED


def _prep_inputs(x, adj, W_proj, attn_src, attn_dst):
    bf = ml_dtypes.bfloat16
    A_blk = np.zeros((IN, 2 * H), np.float32)
    for t in range(H):
        A_blk[t * D:(t + 1) * D, t] = attn_src[t]
        A_blk[t * D:(t + 1) * D, H + t] = attn_dst[t]
    P = W_proj.T.astype(np.float32) @ A_blk                      # [256, 8]
    Wb = W_proj.T.astype(bf)
    Pb = P.astype(bf)
    Pr = (P - Pb.astype(np.float32)).astype(bf)
    wtpb_full = np.concatenate(
        [Wb.astype(np.float32), Pb.astype(np.float32), Pr.astype(np.float32)],
        axis=1).astype(bf)
    wtpb_c = np.ascontiguousarray(wtpb_full.reshape(2, 128, WCOL))

    in_maps = []
    xcache = {}
    for core in range(8):
        b, qq = core // 4, core % 4
        i0 = qq * IBLK
        if b not in xcache:
            xT = np.ascontiguousarray(x[b].T)                    # [256, 4096] f32
            xb_f = xT.astype(bf)
            xr_f = (xT - xb_f.astype(np.float32)).astype(bf)
            xcache[b] = (xT, xb_f, xr_f)
        xT, xb_f, xr_f = xcache[b]
        adjT_c = np.ascontiguousarray(adj[b, i0:i0 + IBLK, :].T.astype(bf))
        in_maps.append({
            "adjT": adjT_c.reshape(CH, 128, IBLK),
            "xb": np.ascontiguousarray(xb_f).reshape(2, 128, N),
            "xr": np.ascontiguousarray(xr_f).reshape(2, 128, N),
            "xob": np.ascontiguousarray(xb_f[:, i0:i0 + IBLK]).reshape(2, 128, IBLK),
            "xor": np.ascontiguousarray(xr_f[:, i0:i0 + IBLK]).reshape(2, 128, IBLK),
            "wtpb": wtpb_c,
        })
    return in_maps


def kernel(x, adj, W_proj, attn_src, attn_dst):
    global LAST_RESULT
    x = np.asarray(x, np.float32)
    adj = np.asarray(adj)
    W_proj = np.asarray(W_proj, np.float32)
    attn_src = np.asarray(attn_src, np.float32)
    attn_dst = np.asarray(attn_dst, np.float32)

    nc = _get_bass()
    in_maps = _prep_inputs(x, adj, W_proj, attn_src, attn_dst)
    br = run_bass_kernel_spmd(nc, in_maps, core_ids=list(range(8)))
    LAST_RESULT = br

    out = np.empty((B, N, H * D), np.float32)
    for core in range(8):
        b, qq = core // 4, core % 4
        i0 = qq * IBLK
        out[b, i0:i0 + IBLK, :] = br.results[core]["outT"].astype(np.float32)
    return out


# revision 45
# speedup vs baseline: 1.0837x; 1.0256x over previous
"""DenseGAT layer on 8 trn2 NeuronCores — transposed-stationary formulation.

Math (per batch b, head t, query node i, source node j):
    z_ij = src_i + dst_j
    W_ij = adj_ij * exp(leakyrelu_0.2(z_ij));  out_i = (W @ h)_i / (W @ 1)_i

Identity: exp(lrelu(z)) = max(e^z, e^{0.2z}), each branch factorizes:
    e^z = e^{src_i} e^{dst_j},  e^{0.2z} = e^{0.2 src_i} e^{0.2 dst_j}
With M1 = 1[z>=0]*adj, b = e^{dst}, d = e^{0.2 dst}, r_i = e^{-0.8 src_i}
(the e^{src_i} row factor cancels in the softmax ratio):
    num   = M1@[b.h|b] + r * (adj@[d.h|d] - M1@[d.h|d])
    out_i = num[0:64] / num[64]

Kernel layout: everything lands i-MAJOR by using the masks/adj as matmul
STATIONARY ([128 j, 128 i] subblocks) and the value matrix V130 =
[b.h | b | d.h | d] (130 cols) as MOVING.  One accumulating pass per head
produces [U_b | den_b | U_d | den_d] at once ([128 i, 130] PSUM per
i-subblock); one shared 260-col pass gives the adj terms for all 4 heads.
LDWEIGHTS of the per-chunk stationaries is fully hidden behind matmuls
(measured 25ns cadence at 2-col moving).  The epilogue is i-major, so the
r_i combine / reciprocal / final scale are per-partition-scalar ops (DVE
reciprocal [128,1]; final multiply via ACT per-partition scale).

Mask build m1 = (src_i + dst_j >= 0) * adjT splits across engines:
DVE fused scalar_tensor_tensor (~1.2us/chunk) for most chunks; for the
rest ACT Sigmoid(1e9*z) (exactly {0,1}, verified) + gpsimd multiply.

h-phase runs in bf16 (4x faster than fp32 matmul); src/dst precision is
restored by sending x and P as bf16 value+residual pairs (xb+xr, Pb+Pr):
sd = xb@Pb + xr@Pb + xb@Pr accumulated in one fp32 PSUM region.

Sharding: core c -> batch c//4, query rows (c%4)*1024..+1024.
"""

import numpy as np
import ml_dtypes
from contextlib import ExitStack

import concourse.bass as bass
import concourse.mybir as mybir
import concourse.tile as tile
from concourse.bass import ts, ds
from concourse.bass_utils import run_bass_kernel_spmd
from concourse.masks import make_identity
from concourse.vector_clock import ScopedClock

B, N, IN = 2, 4096, 256
H, D = 4, 64
IBLK = 1024          # query rows per core
CH = N // 128        # 32 j-chunks
NSUB = IBLK // 128   # 8 i-subblocks per core
OCH = IBLK // 128    # own chunks (i-range) = 8
WCOL = IN + 16       # wtpb cols: [Wb 256 | Pb 8 | Pr 8]
V1 = D + 1           # 65: [value-cols | den-col]

F32 = mybir.dt.float32
BF16 = mybir.dt.bfloat16
FT = mybir.ActivationFunctionType
OP = mybir.AluOpType

LAST_RESULT = None  # BassKernelResults of the most recent run (for test harness)

# mask-build engine assignment per chunk (load balancing across DVE/ACT/GP):
#   GP set: st on ACT (sigmoid), mult on gpsimd
#   ACT-ST set: st on ACT (sigmoid), mult on DVE
#   rest: st + mult both on DVE
GP_SET = ()
ACTST_SET = (1, 2, 4, 5, 8, 10, 11, 13, 14)


def _install_drain_split(maxw=1):
    """This walrus build rejects instructions with more than ~2 sem waits
    ("Too many sync wait commands"). Tile's kernel-tail drain waits on every
    proc's final tick in a single instruction; split it into a chain of SP
    nops carrying one wait each."""
    if getattr(tile.TileContext, "_drain_split_installed", False):
        return

    def _split_drain_and_barrier(self, tick_clock, wait_clock):
        nc = self.nc
        probe = nc.sync.nop(nofuse=True)
        wait_clock.add_sem_waits(probe.ins, ScopedClock({None: tick_clock.global_clock}))
        si = probe.ins.sync_info
        waits = list(si.on_wait) if si is not None else []
        if len(waits) > maxw:
            probe.ins.sync_info = mybir.SyncInfo(
                on_wait=waits[:maxw], on_update=list(si.on_update)
            )
            for i in range(maxw, len(waits), maxw):
                extra = nc.sync.nop(nofuse=True)
                extra.ins.sync_info = mybir.SyncInfo(
                    on_wait=waits[i:i + maxw], on_update=[]
                )
        nc.sync.drain()
        nc.all_engine_barrier()
        assert self.sems is not None
        popped = nc._tile_sem_poison_stack.pop()
        assert popped is self._sem_poison
        nc.clear_and_free_semaphores(list(self.sems.allocated().values()))
        nc.all_engine_barrier()

    tile.TileContext._drain_and_barrier = _split_drain_and_barrier
    tile.TileContext._drain_split_installed = True


def _split_excess_waits(nc, maxw=1):
    """Move excess sem-waits (beyond maxw per instruction) onto same-engine
    NoOps inserted immediately before the instruction."""
    cnt = 0
    tpb = {mybir.EngineType.PE, mybir.EngineType.Activation, mybir.EngineType.Pool,
           mybir.EngineType.DVE, mybir.EngineType.SP}
    for f in nc.m.functions:
        for bb in f.blocks:
            out = []
            changed = False
            for inst in bb.instructions:
                si = getattr(inst, "sync_info", None)
                waits = list(si.on_wait) if si is not None else []
                if len(waits) > maxw and inst.engine in tpb:
                    changed = True
                    nlead = len(waits) - maxw
                    for k in range(0, nlead, maxw):
                        nop = mybir.InstNoOp(
                            name=f"wsplit{cnt}", engine=inst.engine, ins=[], outs=[],
                            sync_info=mybir.SyncInfo(
                                on_wait=waits[k:min(k + maxw, nlead)], on_update=[]))
                        cnt += 1
                        nc.register_instruction(nop, overwrite=True)
                        out.append(nop)
                    inst.sync_info = mybir.SyncInfo(
                        on_wait=waits[nlead:], on_update=list(si.on_update))
                out.append(inst)
            if changed:
                bb.instructions = out
    return cnt


def build_bass():
    _install_drain_split()
    nc = bass.Bass("TRN2", target_bir_lowering=False, debug=False, num_devices=1)

    adjT = nc.dram_tensor("adjT", [CH, 128, IBLK], BF16, kind="ExternalInput")
    xb = nc.dram_tensor("xb", [2, 128, N], BF16, kind="ExternalInput")
    xr = nc.dram_tensor("xr", [2, 128, N], BF16, kind="ExternalInput")
    xob = nc.dram_tensor("xob", [2, 128, IBLK], BF16, kind="ExternalInput")
    xor_ = nc.dram_tensor("xor", [2, 128, IBLK], BF16, kind="ExternalInput")
    wtpb = nc.dram_tensor("wtpb", [2, 128, WCOL], BF16, kind="ExternalInput")
    outT = nc.dram_tensor("outT", [IBLK, H * D], BF16, kind="ExternalOutput")

    with ExitStack() as ctx:
        tc = ctx.enter_context(tile.TileContext(nc))
        const = ctx.enter_context(tc.tile_pool(name="const", bufs=1))

        ident = const.tile([128, 128], F32, tag="ident")
        make_identity(nc, ident[:])

        adjT_sb = const.tile([128, CH, IBLK], BF16, tag="adjT")

        # value matrix per (chunk, head): [b.h | b | d.h | d] (130 cols)
        V = const.tile([128, CH, H, 2 * V1], BF16, tag="V")
        sd_sb = const.tile([128, CH, 8], F32, tag="sd")       # [src 0:4 | dst 4:8]
        ndst = const.tile([128, CH, H], F32, tag="ndst")      # -dst (DVE is_ge scalar)
        pdst9 = const.tile([128, CH, H], F32, tag="pdst9")    # +1e9*dst (ACT bias)
        bcol = const.tile([128, CH, H, 1], F32, tag="bcol")   # e^dst
        dcol = const.tile([128, CH, H, 1], F32, tag="dcol")   # e^{0.2 dst}
        r_sb = const.tile([128, OCH, H], F32, tag="r")        # e^{-0.8 src_own}
        sbb = const.tile([128, H, IBLK], BF16, tag="sbb")     # src_i bcast over j
        srowT = const.tile([32, 128], BF16, tag="srowT")
        A_sb = const.tile([128, NSUB, H * V1], F32, tag="A")  # adj-pass results

        def bcast(dst_ap, src_row_ap):
            # DMA-broadcast one SBUF row across partitions: the repeat is a
            # stride-0 *free* dim on the source, iterated in the same order as
            # the dest's partition dim so the element streams line up.
            lay = [list(src_row_ap.ap[0]), [0, dst_ap.shape[0]]] + [
                list(dims) for dims in src_row_ap.ap[1:]]
            src_b = bass.AP(src_row_ap.tensor, src_row_ap.offset, lay)
            nc.sync.dma_start(dst_ap, src_b)

        # mask pools live across phase A and the masked passes so head-0
        # masks can pre-fill the m1 queue while the h-phase still runs
        m1p = ctx.enter_context(tc.tile_pool(name="m1p", bufs=13))
        stp = ctx.enter_context(tc.tile_pool(name="stp", bufs=4))
        m1_pre = {}

        def emit_mask(t, c):
            m1 = m1p.tile([128, IBLK], BF16, tag="m1")
            kind = (c + 3 * t) % 16
            if kind in GP_SET or kind in ACTST_SET:
                st = stp.tile([128, IBLK], BF16, tag="st")
                nc.scalar.activation(st[:], sbb[:, t, :], FT.Sigmoid,
                                     bias=pdst9[:, c, t:t + 1], scale=1e9)
                if kind in GP_SET:
                    nc.gpsimd.tensor_mul(m1[:], st[:], adjT_sb[:, c, :])
                else:
                    nc.vector.tensor_mul(m1[:], st[:], adjT_sb[:, c, :])
            else:
                nc.vector.scalar_tensor_tensor(m1[:], sbb[:, t, :],
                                               ndst[:, c, t:t + 1],
                                               adjT_sb[:, c, :],
                                               OP.is_ge, OP.mult)
            return m1
            m1 = m1p.tile([128, 1, IBLK], BF16, tag="m1")
            kind = (c + 3 * t) % 16
            if kind in ACTST_SET:
                st = stp.tile([128, IBLK], BF16, tag="st")
                nc.scalar.activation(st[:], sbb[:, t, :], FT.Sigmoid,
                                     bias=pdst9[:, c, t:t + 1], scale=1e9)
                nc.vector.tensor_mul(m1[:, 0, :], st[:], adjT_sb[:, c, :])
            else:
                nc.vector.scalar_tensor_tensor(m1[:, 0, :], sbb[:, t, :],
                                               ndst[:, c, t:t + 1],
                                               adjT_sb[:, c, :],
                                               OP.is_ge, OP.mult)
            return m1

        # ---------------- phase A: h projection + src/dst ----------------
        with (
            tc.tile_pool(name="xin", bufs=1) as xin,
            tc.tile_pool(name="pps", bufs=3, space="PSUM") as pps,
            tc.tile_pool(name="ppt", bufs=1, space="PSUM") as ppt,
        ):
            xb_sb = [xin.tile([128, N], BF16, tag=f"xb{k}", name=f"xbsb{k}") for k in range(2)]
            xr_sb = [xin.tile([128, N], BF16, tag=f"xr{k}", name=f"xrsb{k}") for k in range(2)]
            xob_sb = [xin.tile([128, IBLK], BF16, tag=f"xob{k}", name=f"xobsb{k}") for k in range(2)]
            xor_sb = [xin.tile([128, IBLK], BF16, tag=f"xor{k}", name=f"xorsb{k}") for k in range(2)]
            wtp_sb = [xin.tile([128, WCOL], BF16, tag=f"wt{k}", name=f"wtsb{k}") for k in range(2)]
            # smallest inputs first: sdo/sbb come up within ~8us, h-phase
            # right after; adjT streams behind, hidden under phase A
            for k in range(2):
                nc.sync.dma_start(wtp_sb[k][:], wtpb.ap()[k])
                nc.sync.dma_start(xob_sb[k][:], xob.ap()[k])
                nc.sync.dma_start(xor_sb[k][:], xor_.ap()[k])
            for k in range(2):
                nc.sync.dma_start(xb_sb[k][:], xb.ap()[k])
                nc.sync.dma_start(xr_sb[k][:], xr.ap()[k])
            for c in range(CH):
                nc.sync.dma_start(adjT_sb[:, c, :], adjT.ap()[c])

            # own-row src (fp32-compensated, q-independent via xob/xor inputs)
            sdo_sb = xin.tile([128, OCH, 4], F32, tag="sdo")
            for oc in range(OCH):
                pho = pps.tile([128, 4], F32, tag="pho")
                for k in range(2):
                    nc.tensor.matmul(pho[:], xob_sb[k][:, ts(oc, 128)],
                                     wtp_sb[k][:, IN:IN + 4], start=(k == 0), stop=False)
                    nc.tensor.matmul(pho[:], xob_sb[k][:, ts(oc, 128)],
                                     wtp_sb[k][:, IN + 8:IN + 12], start=False, stop=False)
                    nc.tensor.matmul(pho[:], xor_sb[k][:, ts(oc, 128)],
                                     wtp_sb[k][:, IN:IN + 4], start=False, stop=(k == 1))
                nc.scalar.copy(sdo_sb[:, oc, :], pho[:])
                # r_i = e^{-0.8 src} for own chunks, i-major
                nc.scalar.activation(r_sb[:, oc, :], sdo_sb[:, oc, :], FT.Exp, scale=-0.8)

            # srow: own-chunk src columns -> [32, 128] rows via PE transpose
            stile = xin.tile([128, 32], F32, tag="stile")
            for t in range(H):
                nc.vector.tensor_copy(stile[:, ts(t, OCH)], sdo_sb[:, :, t])
            pst = ppt.tile([32, 128], F32, tag="pst")
            nc.tensor.transpose(pst[:], stile[:], ident[:])
            nc.scalar.copy(srowT[:], pst[:])
            for t in range(H):
                for oc in range(OCH):
                    bcast(sbb[:, t, ts(oc, 128)], srowT[t * OCH + oc:t * OCH + oc + 1, :])

            for c in range(CH):
                ph = pps.tile([128, IN + 8], F32, tag="ph")
                # sd corrections first, full-region matmul last so the whole
                # accumulation group gets a proper stop
                nc.tensor.matmul(ph[:], xb_sb[0][:, ts(c, 128)],
                                 wtp_sb[0][:, 0:IN + 8], start=True, stop=False)
                for k in range(2):
                    # sd += xb@Pr (Pr cols into the same 256:264 psum region)
                    nc.tensor.matmul(ph[:, IN:IN + 8], xb_sb[k][:, ts(c, 128)],
                                     wtp_sb[k][:, IN + 8:IN + 16],
                                     start=False, stop=False)
                    # sd += xr@Pb
                    nc.tensor.matmul(ph[:, IN:IN + 8], xr_sb[k][:, ts(c, 128)],
                                     wtp_sb[k][:, IN:IN + 8],
                                     start=False, stop=False)
                nc.tensor.matmul(ph[:], xb_sb[1][:, ts(c, 128)],
                                 wtp_sb[1][:, 0:IN + 8], start=False, stop=True)
                # drains: sd on ACT + per-chunk exps; V built straight from
                # PSUM on DVE (b-branch) — gp can't read PSUM, so d-branch too
                nc.scalar.copy(sd_sb[:, c, :], ph[:, IN:IN + 8])
                nc.scalar.activation(bcol[:, c, :, 0], sd_sb[:, c, 4:8], FT.Exp)
                nc.scalar.activation(dcol[:, c, :, 0], sd_sb[:, c, 4:8],
                                     FT.Exp, scale=0.2)
                php = ph[:, 0:IN]
                hview = bass.AP(php.tensor, php.offset,
                                [list(php.ap[0]), [D, H], [1, D]])
                _, cb = bass.broadcast_tensor_aps(
                    V[:, c, :, 0:D], bcol[:, c, :, :])
                nc.vector.tensor_tensor(V[:, c, :, 0:D], hview, cb, OP.mult)
                _, cd = bass.broadcast_tensor_aps(
                    V[:, c, :, V1:V1 + D], dcol[:, c, :, :])
                nc.vector.tensor_tensor(V[:, c, :, V1:V1 + D], hview, cd, OP.mult)
                if c % 8 == 7:
                    g = ds(c - 7, 8)
                    nc.scalar.activation(ndst[:, g, :], sd_sb[:, g, 4:8],
                                         FT.Copy, scale=-1.0)
                    nc.scalar.activation(pdst9[:, g, :], sd_sb[:, g, 4:8],
                                         FT.Copy, scale=1e9)
                    for t in range(H):
                        nc.vector.tensor_copy(V[:, g, t, D], bcol[:, g, t, 0])
                        nc.vector.tensor_copy(V[:, g, t, V1 + D], dcol[:, g, t, 0])
                    for cc in range(c - 7, c + 1):
                        if cc < 8:  # leave m1-pool slack so later phase-A
                            m1_pre[(0, cc)] = emit_mask(0, cc)  # DVE work isn't blocked

        # ------------- masked passes per head (adj rides with head 0) ------
        with (
            tc.tile_pool(name="pm", bufs=1, space="PSUM") as pm,
            tc.tile_pool(name="psb", bufs=2) as psb,
            tc.tile_pool(name="epp", bufs=2) as epp,
        ):
            def epilogue(t, Ps):
                # batched where the scalar is shared; per-s where r/rec differ
                t1 = epp.tile([128, NSUB, V1], F32, tag="t1")
                nc.gpsimd.tensor_tensor(t1[:], A_sb[:, :, ts(t, V1)],
                                        Ps[:, :, V1:2 * V1], OP.subtract)
                for s in range(NSUB):
                    nc.scalar.activation(t1[:, s, :], t1[:, s, :], FT.Copy,
                                         scale=r_sb[:, s, t:t + 1])
                o2 = epp.tile([128, NSUB, V1], F32, tag="o2")
                nc.vector.tensor_tensor(o2[:], t1[:], Ps[:, :, 0:V1], OP.add)
                rec = epp.tile([128, NSUB], F32, tag="rec")
                nc.vector.reciprocal(rec[:], o2[:, :, D])
                for s in range(NSUB):
                    of = epp.tile([128, D], BF16, tag="of")
                    nc.scalar.activation(of[:], o2[:, s, 0:D], FT.Copy,
                                         scale=rec[:, s:s + 1])
                    nc.sync.dma_start(outT.ap()[ds(s * 128, 128), ts(t, D)], of[:])

            prev = None  # (t, Ps) pending epilogue, emitted a few chunks into
            # the next head so its ops never block that head's mask queue
            for t in range(H):
                # P tile: 8 subblocks, one full PSUM bank (2KB zero-region)
                # each; masked pass in cols 0:130, and for head 0 the shared
                # adj pass accumulates in cols 130:390 of the same banks
                P = pm.tile([128, NSUB, 512], F32, tag="P")
                for c in range(CH):
                    m1 = m1_pre.pop((t, c), None)
                    if m1 is None:
                        m1 = emit_mask(t, c)
                    for s in range(NSUB):
                        nc.tensor.matmul(P[:, s, 0:2 * V1], m1[:, ts(s, 128)],
                                         V[:, c, t, :],
                                         start=(c == 0), stop=(c == CH - 1))
                        if t == 0:
                            nc.tensor.matmul(P[:, s, 2 * V1:6 * V1],
                                             adjT_sb[:, c, ts(s, 128)],
                                             V[:, c, :, V1:2 * V1],
                                             start=False, stop=False,
                                             skip_group_check=True)
                    if prev is not None and c == 3:
                        epilogue(*prev)
                        prev = None
                # drain PSUM fast (frees P for the next head)
                Ps = psb.tile([128, NSUB, 2 * V1], F32, tag="Ps")
                if t == 0:
                    nc.scalar.copy(A_sb[:, :, :], P[:, :, 2 * V1:6 * V1])
                nc.scalar.copy(Ps[:, :, :], P[:, :, 0:2 * V1])
                prev = (t, Ps)
            epilogue(*prev)
    _split_excess_waits(nc)
    return nc


_CACHED = None


def _get_bass():
    global _CACHED
    if _CACHED is None:
        _CACHED = build_bass()
    return _CACHED


def _prep_inputs(x, adj, W_proj, attn_src, attn_dst):
    bf = ml_dtypes.bfloat16
    A_blk = np.zeros((IN, 2 * H), np.float32)
    for t in range(H):
        A_blk[t * D:(t + 1) * D, t] = attn_src[t]
        A_blk[t * D:(t + 1) * D, H + t] = attn_dst[t]
    P = W_proj.T.astype(np.float32) @ A_blk                      # [256, 8]
    Wb = W_proj.T.astype(bf)
    Pb = P.astype(bf)
    Pr = (P - Pb.astype(np.float32)).astype(bf)
    wtpb_full = np.concatenate(
        [Wb.astype(np.float32), Pb.astype(np.float32), Pr.astype(np.float32)],
        axis=1).astype(bf)
    wtpb_c = np.ascontiguousarray(wtpb_full.reshape(2, 128, WCOL))

    in_maps = []
    xcache = {}
    for core in range(8):
        b, qq = core // 4, core % 4
        i0 = qq * IBLK
        if b not in xcache:
            xT = np.ascontiguousarray(x[b].T)                    # [256, 4096] f32
            xb_f = xT.astype(bf)
            xr_f = (xT - xb_f.astype(np.float32)).astype(bf)
            xcache[b] = (xT, xb_f, xr_f)
        xT, xb_f, xr_f = xcache[b]
        adjT_c = np.ascontiguousarray(adj[b, i0:i0 + IBLK, :].T.astype(bf))
        in_maps.append({
            "adjT": adjT_c.reshape(CH, 128, IBLK),
            "xb": np.ascontiguousarray(xb_f).reshape(2, 128, N),
            "xr": np.ascontiguousarray(xr_f).reshape(2, 128, N),
            "xob": np.ascontiguousarray(xb_f[:, i0:i0 + IBLK]).reshape(2, 128, IBLK),
            "xor": np.ascontiguousarray(xr_f[:, i0:i0 + IBLK]).reshape(2, 128, IBLK),
            "wtpb": wtpb_c,
        })
    return in_maps


def kernel(x, adj, W_proj, attn_src, attn_dst):
    global LAST_RESULT
    x = np.asarray(x, np.float32)
    adj = np.asarray(adj)
    W_proj = np.asarray(W_proj, np.float32)
    attn_src = np.asarray(attn_src, np.float32)
    attn_dst = np.asarray(attn_dst, np.float32)

    nc = _get_bass()
    in_maps = _prep_inputs(x, adj, W_proj, attn_src, attn_dst)
    br = run_bass_kernel_spmd(nc, in_maps, core_ids=list(range(8)))
    LAST_RESULT = br

    out = np.empty((B, N, H * D), np.float32)
    for core in range(8):
        b, qq = core // 4, core % 4
        i0 = qq * IBLK
        out[b, i0:i0 + IBLK, :] = br.results[core]["outT"].astype(np.float32)
    return out


# revision 46
# speedup vs baseline: 1.0870x; 1.0031x over previous
"""DenseGAT layer on 8 trn2 NeuronCores — transposed-stationary formulation.

Math (per batch b, head t, query node i, source node j):
    z_ij = src_i + dst_j
    W_ij = adj_ij * exp(leakyrelu_0.2(z_ij));  out_i = (W @ h)_i / (W @ 1)_i

Identity: exp(lrelu(z)) = max(e^z, e^{0.2z}), each branch factorizes:
    e^z = e^{src_i} e^{dst_j},  e^{0.2z} = e^{0.2 src_i} e^{0.2 dst_j}
With M1 = 1[z>=0]*adj, b = e^{dst}, d = e^{0.2 dst}, r_i = e^{-0.8 src_i}
(the e^{src_i} row factor cancels in the softmax ratio):
    num   = M1@[b.h|b] + r * (adj@[d.h|d] - M1@[d.h|d])
    out_i = num[0:64] / num[64]

Kernel layout: everything lands i-MAJOR by using the masks/adj as matmul
STATIONARY ([128 j, 128 i] subblocks) and the value matrix V130 =
[b.h | b | d.h | d] (130 cols) as MOVING.  One accumulating pass per head
produces [U_b | den_b | U_d | den_d] at once ([128 i, 130] PSUM per
i-subblock); one shared 260-col pass gives the adj terms for all 4 heads.
LDWEIGHTS of the per-chunk stationaries is fully hidden behind matmuls
(measured 25ns cadence at 2-col moving).  The epilogue is i-major, so the
r_i combine / reciprocal / final scale are per-partition-scalar ops (DVE
reciprocal [128,1]; final multiply via ACT per-partition scale).

Mask build m1 = (src_i + dst_j >= 0) * adjT splits across engines:
DVE fused scalar_tensor_tensor (~1.2us/chunk) for most chunks; for the
rest ACT Sigmoid(1e9*z) (exactly {0,1}, verified) + gpsimd multiply.

h-phase runs in bf16 (4x faster than fp32 matmul); src/dst precision is
restored by sending x and P as bf16 value+residual pairs (xb+xr, Pb+Pr):
sd = xb@Pb + xr@Pb + xb@Pr accumulated in one fp32 PSUM region.

Sharding: core c -> batch c//4, query rows (c%4)*1024..+1024.
"""

import numpy as np
import ml_dtypes
from contextlib import ExitStack

import concourse.bass as bass
import concourse.mybir as mybir
import concourse.tile as tile
from concourse.bass import ts, ds
from concourse.bass_utils import run_bass_kernel_spmd
from concourse.masks import make_identity
from concourse.vector_clock import ScopedClock

B, N, IN = 2, 4096, 256
H, D = 4, 64
IBLK = 1024          # query rows per core
CH = N // 128        # 32 j-chunks
NSUB = IBLK // 128   # 8 i-subblocks per core
OCH = IBLK // 128    # own chunks (i-range) = 8
WCOL = IN + 16       # wtpb cols: [Wb 256 | Pb 8 | Pr 8]
V1 = D + 1           # 65: [value-cols | den-col]

F32 = mybir.dt.float32
BF16 = mybir.dt.bfloat16
FT = mybir.ActivationFunctionType
OP = mybir.AluOpType

LAST_RESULT = None  # BassKernelResults of the most recent run (for test harness)

# mask-build engine assignment per chunk (load balancing across DVE/ACT/GP):
#   GP set: st on ACT (sigmoid), mult on gpsimd
#   ACT-ST set: st on ACT (sigmoid), mult on DVE
#   rest: st + mult both on DVE
GP_SET = ()
ACTST_SET = (1, 2, 4, 5, 8, 10, 11, 13, 14)


def _install_drain_split(maxw=1):
    """This walrus build rejects instructions with more than ~2 sem waits
    ("Too many sync wait commands"). Tile's kernel-tail drain waits on every
    proc's final tick in a single instruction; split it into a chain of SP
    nops carrying one wait each."""
    if getattr(tile.TileContext, "_drain_split_installed", False):
        return

    def _split_drain_and_barrier(self, tick_clock, wait_clock):
        nc = self.nc
        probe = nc.sync.nop(nofuse=True)
        wait_clock.add_sem_waits(probe.ins, ScopedClock({None: tick_clock.global_clock}))
        si = probe.ins.sync_info
        waits = list(si.on_wait) if si is not None else []
        if len(waits) > maxw:
            probe.ins.sync_info = mybir.SyncInfo(
                on_wait=waits[:maxw], on_update=list(si.on_update)
            )
            for i in range(maxw, len(waits), maxw):
                extra = nc.sync.nop(nofuse=True)
                extra.ins.sync_info = mybir.SyncInfo(
                    on_wait=waits[i:i + maxw], on_update=[]
                )
        nc.sync.drain()
        nc.all_engine_barrier()
        assert self.sems is not None
        popped = nc._tile_sem_poison_stack.pop()
        assert popped is self._sem_poison
        nc.clear_and_free_semaphores(list(self.sems.allocated().values()))
        nc.all_engine_barrier()

    tile.TileContext._drain_and_barrier = _split_drain_and_barrier
    tile.TileContext._drain_split_installed = True


def _split_excess_waits(nc, maxw=1):
    """Move excess sem-waits (beyond maxw per instruction) onto same-engine
    NoOps inserted immediately before the instruction."""
    cnt = 0
    tpb = {mybir.EngineType.PE, mybir.EngineType.Activation, mybir.EngineType.Pool,
           mybir.EngineType.DVE, mybir.EngineType.SP}
    for f in nc.m.functions:
        for bb in f.blocks:
            out = []
            changed = False
            for inst in bb.instructions:
                si = getattr(inst, "sync_info", None)
                waits = list(si.on_wait) if si is not None else []
                if len(waits) > maxw and inst.engine in tpb:
                    changed = True
                    nlead = len(waits) - maxw
                    for k in range(0, nlead, maxw):
                        nop = mybir.InstNoOp(
                            name=f"wsplit{cnt}", engine=inst.engine, ins=[], outs=[],
                            sync_info=mybir.SyncInfo(
                                on_wait=waits[k:min(k + maxw, nlead)], on_update=[]))
                        cnt += 1
                        nc.register_instruction(nop, overwrite=True)
                        out.append(nop)
                    inst.sync_info = mybir.SyncInfo(
                        on_wait=waits[nlead:], on_update=list(si.on_update))
                out.append(inst)
            if changed:
                bb.instructions = out
    return cnt


def build_bass():
    _install_drain_split()
    nc = bass.Bass("TRN2", target_bir_lowering=False, debug=False, num_devices=1)

    adjT = nc.dram_tensor("adjT", [CH, 128, IBLK], BF16, kind="ExternalInput")
    xb = nc.dram_tensor("xb", [2, 128, N], BF16, kind="ExternalInput")
    xr = nc.dram_tensor("xr", [2, 128, N], BF16, kind="ExternalInput")
    xob = nc.dram_tensor("xob", [2, 128, IBLK], BF16, kind="ExternalInput")
    xor_ = nc.dram_tensor("xor", [2, 128, IBLK], BF16, kind="ExternalInput")
    wtpb = nc.dram_tensor("wtpb", [2, 128, WCOL], BF16, kind="ExternalInput")
    outT = nc.dram_tensor("outT", [IBLK, H * D], BF16, kind="ExternalOutput")

    with ExitStack() as ctx:
        tc = ctx.enter_context(tile.TileContext(nc))
        const = ctx.enter_context(tc.tile_pool(name="const", bufs=1))

        ident = const.tile([128, 128], F32, tag="ident")
        make_identity(nc, ident[:])

        adjT_sb = const.tile([128, CH, IBLK], BF16, tag="adjT")

        # value matrix per (chunk, head): [b.h | b | d.h | d] (130 cols)
        V = const.tile([128, CH, H, 2 * V1], BF16, tag="V")
        sd_sb = const.tile([128, CH, 8], F32, tag="sd")       # [src 0:4 | dst 4:8]
        ndst = const.tile([128, CH, H], F32, tag="ndst")      # -dst (DVE is_ge scalar)
        pdst9 = const.tile([128, CH, H], F32, tag="pdst9")    # +1e9*dst (ACT bias)
        bcol = const.tile([128, CH, H, 1], F32, tag="bcol")   # e^dst
        dcol = const.tile([128, CH, H, 1], F32, tag="dcol")   # e^{0.2 dst}
        r_sb = const.tile([128, OCH, H], F32, tag="r")        # e^{-0.8 src_own}
        sbb = const.tile([128, H, IBLK], BF16, tag="sbb")     # src_i bcast over j
        srowT = const.tile([32, 128], BF16, tag="srowT")
        A_sb = const.tile([128, NSUB, H * V1], F32, tag="A")  # adj-pass results

        def bcast(dst_ap, src_row_ap):
            # DMA-broadcast one SBUF row across partitions: the repeat is a
            # stride-0 *free* dim on the source, iterated in the same order as
            # the dest's partition dim so the element streams line up.
            lay = [list(src_row_ap.ap[0]), [0, dst_ap.shape[0]]] + [
                list(dims) for dims in src_row_ap.ap[1:]]
            src_b = bass.AP(src_row_ap.tensor, src_row_ap.offset, lay)
            nc.sync.dma_start(dst_ap, src_b)

        # mask pools live across phase A and the masked passes so head-0
        # masks can pre-fill the m1 queue while the h-phase still runs
        m1p = ctx.enter_context(tc.tile_pool(name="m1p", bufs=14))
        stp = ctx.enter_context(tc.tile_pool(name="stp", bufs=4))
        m1_pre = {}

        def emit_mask(t, c):
            m1 = m1p.tile([128, IBLK], BF16, tag="m1")
            kind = (c + 3 * t) % 16
            if kind in GP_SET or kind in ACTST_SET:
                st = stp.tile([128, IBLK], BF16, tag="st")
                nc.scalar.activation(st[:], sbb[:, t, :], FT.Sigmoid,
                                     bias=pdst9[:, c, t:t + 1], scale=1e9)
                if kind in GP_SET:
                    nc.gpsimd.tensor_mul(m1[:], st[:], adjT_sb[:, c, :])
                else:
                    nc.vector.tensor_mul(m1[:], st[:], adjT_sb[:, c, :])
            else:
                nc.vector.scalar_tensor_tensor(m1[:], sbb[:, t, :],
                                               ndst[:, c, t:t + 1],
                                               adjT_sb[:, c, :],
                                               OP.is_ge, OP.mult)
            return m1
            m1 = m1p.tile([128, 1, IBLK], BF16, tag="m1")
            kind = (c + 3 * t) % 16
            if kind in ACTST_SET:
                st = stp.tile([128, IBLK], BF16, tag="st")
                nc.scalar.activation(st[:], sbb[:, t, :], FT.Sigmoid,
                                     bias=pdst9[:, c, t:t + 1], scale=1e9)
                nc.vector.tensor_mul(m1[:, 0, :], st[:], adjT_sb[:, c, :])
            else:
                nc.vector.scalar_tensor_tensor(m1[:, 0, :], sbb[:, t, :],
                                               ndst[:, c, t:t + 1],
                                               adjT_sb[:, c, :],
                                               OP.is_ge, OP.mult)
            return m1

        # ---------------- phase A: h projection + src/dst ----------------
        with (
            tc.tile_pool(name="xin", bufs=1) as xin,
            tc.tile_pool(name="pps", bufs=3, space="PSUM") as pps,
            tc.tile_pool(name="ppt", bufs=1, space="PSUM") as ppt,
        ):
            xb_sb = [xin.tile([128, N], BF16, tag=f"xb{k}", name=f"xbsb{k}") for k in range(2)]
            xr_sb = [xin.tile([128, N], BF16, tag=f"xr{k}", name=f"xrsb{k}") for k in range(2)]
            xob_sb = [xin.tile([128, IBLK], BF16, tag=f"xob{k}", name=f"xobsb{k}") for k in range(2)]
            xor_sb = [xin.tile([128, IBLK], BF16, tag=f"xor{k}", name=f"xorsb{k}") for k in range(2)]
            wtp_sb = [xin.tile([128, WCOL], BF16, tag=f"wt{k}", name=f"wtsb{k}") for k in range(2)]
            # smallest inputs first: sdo/sbb come up within ~8us, h-phase
            # right after; adjT streams behind, hidden under phase A
            for k in range(2):
                nc.sync.dma_start(wtp_sb[k][:], wtpb.ap()[k])
                nc.sync.dma_start(xob_sb[k][:], xob.ap()[k])
                nc.sync.dma_start(xor_sb[k][:], xor_.ap()[k])
            for k in range(2):
                nc.sync.dma_start(xb_sb[k][:], xb.ap()[k])
                nc.sync.dma_start(xr_sb[k][:], xr.ap()[k])
            for c in range(CH):
                nc.sync.dma_start(adjT_sb[:, c, :], adjT.ap()[c])

            # own-row src (fp32-compensated, q-independent via xob/xor inputs)
            sdo_sb = xin.tile([128, OCH, 4], F32, tag="sdo")
            for oc in range(OCH):
                pho = pps.tile([128, 4], F32, tag="pho")
                for k in range(2):
                    nc.tensor.matmul(pho[:], xob_sb[k][:, ts(oc, 128)],
                                     wtp_sb[k][:, IN:IN + 4], start=(k == 0), stop=False)
                    nc.tensor.matmul(pho[:], xob_sb[k][:, ts(oc, 128)],
                                     wtp_sb[k][:, IN + 8:IN + 12], start=False, stop=False)
                    nc.tensor.matmul(pho[:], xor_sb[k][:, ts(oc, 128)],
                                     wtp_sb[k][:, IN:IN + 4], start=False, stop=(k == 1))
                nc.scalar.copy(sdo_sb[:, oc, :], pho[:])
                # r_i = e^{-0.8 src} for own chunks, i-major
                nc.scalar.activation(r_sb[:, oc, :], sdo_sb[:, oc, :], FT.Exp, scale=-0.8)

            # srow: own-chunk src columns -> [32, 128] rows via PE transpose
            stile = xin.tile([128, 32], F32, tag="stile")
            for t in range(H):
                nc.vector.tensor_copy(stile[:, ts(t, OCH)], sdo_sb[:, :, t])
            pst = ppt.tile([32, 128], F32, tag="pst")
            nc.tensor.transpose(pst[:], stile[:], ident[:])
            nc.scalar.copy(srowT[:], pst[:])
            for t in range(H):
                for oc in range(OCH):
                    bcast(sbb[:, t, ts(oc, 128)], srowT[t * OCH + oc:t * OCH + oc + 1, :])

            for c in range(CH):
                ph = pps.tile([128, IN + 8], F32, tag="ph")
                # sd corrections first, full-region matmul last so the whole
                # accumulation group gets a proper stop
                nc.tensor.matmul(ph[:], xb_sb[0][:, ts(c, 128)],
                                 wtp_sb[0][:, 0:IN + 8], start=True, stop=False)
                for k in range(2):
                    # sd += xb@Pr (Pr cols into the same 256:264 psum region)
                    nc.tensor.matmul(ph[:, IN:IN + 8], xb_sb[k][:, ts(c, 128)],
                                     wtp_sb[k][:, IN + 8:IN + 16],
                                     start=False, stop=False)
                    # sd += xr@Pb
                    nc.tensor.matmul(ph[:, IN:IN + 8], xr_sb[k][:, ts(c, 128)],
                                     wtp_sb[k][:, IN:IN + 8],
                                     start=False, stop=False)
                nc.tensor.matmul(ph[:], xb_sb[1][:, ts(c, 128)],
                                 wtp_sb[1][:, 0:IN + 8], start=False, stop=True)
                # drains: sd on ACT + per-chunk exps; V built straight from
                # PSUM on DVE (b-branch) — gp can't read PSUM, so d-branch too
                nc.scalar.copy(sd_sb[:, c, :], ph[:, IN:IN + 8])
                nc.scalar.activation(bcol[:, c, :, 0], sd_sb[:, c, 4:8], FT.Exp)
                nc.scalar.activation(dcol[:, c, :, 0], sd_sb[:, c, 4:8],
                                     FT.Exp, scale=0.2)
                php = ph[:, 0:IN]
                hview = bass.AP(php.tensor, php.offset,
                                [list(php.ap[0]), [D, H], [1, D]])
                _, cb = bass.broadcast_tensor_aps(
                    V[:, c, :, 0:D], bcol[:, c, :, :])
                nc.vector.tensor_tensor(V[:, c, :, 0:D], hview, cb, OP.mult)
                _, cd = bass.broadcast_tensor_aps(
                    V[:, c, :, V1:V1 + D], dcol[:, c, :, :])
                nc.vector.tensor_tensor(V[:, c, :, V1:V1 + D], hview, cd, OP.mult)
                if c % 8 == 7:
                    g = ds(c - 7, 8)
                    nc.scalar.activation(ndst[:, g, :], sd_sb[:, g, 4:8],
                                         FT.Copy, scale=-1.0)
                    nc.scalar.activation(pdst9[:, g, :], sd_sb[:, g, 4:8],
                                         FT.Copy, scale=1e9)
                    for t in range(H):
                        nc.vector.tensor_copy(V[:, g, t, D], bcol[:, g, t, 0])
                        nc.vector.tensor_copy(V[:, g, t, V1 + D], dcol[:, g, t, 0])
                    for cc in range(c - 7, c + 1):
                        if cc < 8:  # leave m1-pool slack so later phase-A
                            m1_pre[(0, cc)] = emit_mask(0, cc)  # DVE work isn't blocked

        # ------------- masked passes per head (adj rides with head 0) ------
        with (
            tc.tile_pool(name="pm", bufs=1, space="PSUM") as pm,
            tc.tile_pool(name="psb", bufs=2) as psb,
            tc.tile_pool(name="epp", bufs=2) as epp,
        ):
            def epilogue(t, Ps):
                # batched where the scalar is shared; per-s where r/rec differ
                t1 = epp.tile([128, NSUB, V1], F32, tag="t1")
                nc.gpsimd.tensor_tensor(t1[:], A_sb[:, :, ts(t, V1)],
                                        Ps[:, :, V1:2 * V1], OP.subtract)
                for s in range(NSUB):
                    nc.scalar.activation(t1[:, s, :], t1[:, s, :], FT.Copy,
                                         scale=r_sb[:, s, t:t + 1])
                o2 = epp.tile([128, NSUB, V1], F32, tag="o2")
                nc.vector.tensor_tensor(o2[:], t1[:], Ps[:, :, 0:V1], OP.add)
                rec = epp.tile([128, NSUB], F32, tag="rec")
                nc.vector.reciprocal(rec[:], o2[:, :, D])
                for s in range(NSUB):
                    of = epp.tile([128, D], BF16, tag="of")
                    nc.scalar.activation(of[:], o2[:, s, 0:D], FT.Copy,
                                         scale=rec[:, s:s + 1])
                    nc.sync.dma_start(outT.ap()[ds(s * 128, 128), ts(t, D)], of[:])

            prev = None  # (t, Ps) pending epilogue, emitted a few chunks into
            # the next head so its ops never block that head's mask queue
            for t in range(H):
                # P tile: 8 subblocks, one full PSUM bank (2KB zero-region)
                # each; masked pass in cols 0:130, and for head 0 the shared
                # adj pass accumulates in cols 130:390 of the same banks
                P = pm.tile([128, NSUB, 512], F32, tag="P")
                for c in range(CH):
                    m1 = m1_pre.pop((t, c), None)
                    if m1 is None:
                        m1 = emit_mask(t, c)
                    for s in range(NSUB):
                        nc.tensor.matmul(P[:, s, 0:2 * V1], m1[:, ts(s, 128)],
                                         V[:, c, t, :],
                                         start=(c == 0), stop=(c == CH - 1))
                        if t == 0:
                            nc.tensor.matmul(P[:, s, 2 * V1:6 * V1],
                                             adjT_sb[:, c, ts(s, 128)],
                                             V[:, c, :, V1:2 * V1],
                                             start=False, stop=False,
                                             skip_group_check=True)
                    if prev is not None and c == 3:
                        epilogue(*prev)
                        prev = None
                # drain PSUM fast (frees P for the next head)
                Ps = psb.tile([128, NSUB, 2 * V1], F32, tag="Ps")
                if t == 0:
                    nc.scalar.copy(A_sb[:, :, :], P[:, :, 2 * V1:6 * V1])
                nc.scalar.copy(Ps[:, :, :], P[:, :, 0:2 * V1])
                prev = (t, Ps)
            epilogue(*prev)
    _split_excess_waits(nc)
    return nc


_CACHED = None


def _get_bass():
    global _CACHED
    if _CACHED is None:
        _CACHED = build_bass()
    return _CACHED


def _prep_inputs(x, adj, W_proj, attn_src, attn_dst):
    bf = ml_dtypes.bfloat16
    A_blk = np.zeros((IN, 2 * H), np.float32)
    for t in range(H):
        A_blk[t * D:(t + 1) * D, t] = attn_src[t]
        A_blk[t * D:(t + 1) * D, H + t] = attn_dst[t]
    P = W_proj.T.astype(np.float32) @ A_blk                      # [256, 8]
    Wb = W_proj.T.astype(bf)
    Pb = P.astype(bf)
    Pr = (P - Pb.astype(np.float32)).astype(bf)
    wtpb_full = np.concatenate(
        [Wb.astype(np.float32), Pb.astype(np.float32), Pr.astype(np.float32)],
        axis=1).astype(bf)
    wtpb_c = np.ascontiguousarray(wtpb_full.reshape(2, 128, WCOL))

    in_maps = []
    xcache = {}
    for core in range(8):
        b, qq = core // 4, core % 4
        i0 = qq * IBLK
        if b not in xcache:
            xT = np.ascontiguousarray(x[b].T)                    # [256, 4096] f32
            xb_f = xT.astype(bf)
            xr_f = (xT - xb_f.astype(np.float32)).astype(bf)
            xcache[b] = (xT, xb_f, xr_f)
        xT, xb_f, xr_f = xcache[b]
        adjT_c = np.ascontiguousarray(adj[b, i0:i0 + IBLK, :].T.astype(bf))
        in_maps.append({
            "adjT": adjT_c.reshape(CH, 128, IBLK),
            "xb": np.ascontiguousarray(xb_f).reshape(2, 128, N),
            "xr": np.ascontiguousarray(xr_f).reshape(2, 128, N),
            "xob": np.ascontiguousarray(xb_f[:, i0:i0 + IBLK]).reshape(2, 128, IBLK),
            "xor": np.ascontiguousarray(xr_f[:, i0:i0 + IBLK]).reshape(2, 128, IBLK),
            "wtpb": wtpb_c,
        })
    return in_maps


def kernel(x, adj, W_proj, attn_src, attn_dst):
    global LAST_RESULT
    x = np.asarray(x, np.float32)
    adj = np.asarray(adj)
    W_proj = np.asarray(W_proj, np.float32)
    attn_src = np.asarray(attn_src, np.float32)
    attn_dst = np.asarray(attn_dst, np.float32)

    nc = _get_bass()
    in_maps = _prep_inputs(x, adj, W_proj, attn_src, attn_dst)
    br = run_bass_kernel_spmd(nc, in_maps, core_ids=list(range(8)))
    LAST_RESULT = br

    out = np.empty((B, N, H * D), np.float32)
    for core in range(8):
        b, qq = core // 4, core % 4
        i0 = qq * IBLK
        out[b, i0:i0 + IBLK, :] = br.results[core]["outT"].astype(np.float32)
    return out
